# revision 2
# baseline (speedup 1.0000x reference)
"""Trainium2 Bass kernel for nn_LoRALinear (DoRA-style LoRA linear).

Reference math:
    base = x @ W^T
    lora = sc * (x @ A^T) @ B^T          (sc = 2.0)
    w_eff = W + sc * (B @ A)
    s = magnitude / ||w_eff||_row         (row norm over in_dim)
    out = base + (s - 1) * base + s * lora
        = s * (base + lora)
        = x @ (s[:, None] * w_eff)^T

The whole op collapses to one dense matmul with a derived weight.

The derived weight (w_eff, its row norms, the DoRA scale s) depends only
on the small weight tensors, so it is folded on the host: W_fin =
(s ⊙ w_eff)^T, cast to bf16.  x is cast to bf16 and pre-transposed per
core on the host, so the device does nothing but the main matmul:
32 m-tiles x 16 accumulating bf16 matmuls (8 k-tiles x 2 n-halves),
draining PSUM to fp16 and DMAing out.  bf16 multiplies with fp32 PSUM
accumulation give ~2e-3 max rel err (tolerance 2e-2).

Data-parallel over tokens: each of the 8 cores owns 4096 of the 32768
rows.  PE floor: 512 matmuls x 512 cols / 2.4 GHz ~= 110 us/core.
"""

import os
import numpy as np
import ml_dtypes
from contextlib import ExitStack

import concourse.bass as bass
import concourse.mybir as mybir
import concourse.tile as tile
from concourse import bacc
from concourse.bass import ts
from concourse.bass_utils import run_bass_kernel_spmd

N_CORES = 8
B, S, D_IN, D_OUT, R = 4, 8192, 1024, 1024, 16
SCALING = 32.0 / 16.0
M_TOT = B * S                 # 32768 tokens
M_CORE = M_TOT // N_CORES     # 4096 tokens per core
P = 128
K_TILES = D_IN // P           # 8
MG = 512                      # tokens per x DMA group
N_GROUPS = M_CORE // MG       # 8
MT_PER_G = MG // P            # 4 m-tiles per group
NH = D_OUT // 512             # 2 n-halves of 512
F32 = mybir.dt.float32
F16 = mybir.dt.float16
BF16 = mybir.dt.bfloat16


def _kernel_body(ctx: ExitStack, tc: "tile.TileContext", xT, wT, out):
    nc = tc.nc
    w_pool = ctx.enter_context(tc.tile_pool(name="w", bufs=1))
    xt_pool = ctx.enter_context(tc.tile_pool(name="xt", bufs=3))
    o_pool = ctx.enter_context(tc.tile_pool(name="o", bufs=4))
    ps_out = ctx.enter_context(tc.tile_pool(name="ps_out", bufs=4, space="PSUM"))

    # Replicated final weight, k-tiles side by side: [128, 8*1024] bf16.
    wsb = w_pool.tile([P, K_TILES * D_OUT], BF16)
    for kt in range(K_TILES):
        nc.sync.dma_start(wsb[:, ts(kt, D_OUT)], wT[ts(kt, P), :])

    for g in range(N_GROUPS):
        # x k-tiles for 512 tokens, side by side: [128, 8*512] bf16.
        xtg = xt_pool.tile([P, K_TILES * MG], BF16, tag="xt")
        for kt in range(K_TILES):
            nc.sync.dma_start(xtg[:, ts(kt, MG)], xT[ts(kt, P), ts(g, MG)])

        for mt in range(MT_PER_G):
            m = g * MT_PER_G + mt
            psos = [
                ps_out.tile([P, 512], F32, tag="out", name=f"pso{h}") for h in range(NH)
            ]
            # kt outer so the very first matmul only needs w k-tile 0 and
            # this group's x; later w/x DMAs land while the PE streams.
            for kt in range(K_TILES):
                xsl = xtg[:, kt * MG + mt * P : kt * MG + (mt + 1) * P]
                for h in range(NH):
                    nc.tensor.matmul(
                        psos[h][:],
                        lhsT=xsl,
                        rhs=wsb[:, kt * D_OUT + h * 512 : kt * D_OUT + (h + 1) * 512],
                        start=(kt == 0),
                        stop=(kt == K_TILES - 1),
                    )
            o_sb = o_pool.tile([P, D_OUT], F16, tag="o")
            for h in range(NH):
                nc.scalar.copy(o_sb[:, ts(h, 512)], psos[h][:])
            nc.sync.dma_start(out[ts(m, P), :], o_sb[:])


def build_nc() -> "bass.Bass":
    nc = bacc.Bacc(
        "TRN2",
        target_bir_lowering=False,
        debug=False,
        num_devices=N_CORES,
    )
    xT = nc.dram_tensor("xT", [D_IN, M_CORE], BF16, kind="ExternalInput").ap()
    wT = nc.dram_tensor("wT", [D_IN, D_OUT], BF16, kind="ExternalInput").ap()
    out = nc.dram_tensor("out", [M_CORE, D_OUT], F16, kind="ExternalOutput").ap()

    with tile.TileContext(nc) as tc, ExitStack() as ctx:
        _kernel_body(ctx, tc, xT, wT, out)
    nc.compile()
    return nc


_NC_CACHE: list = []


def get_nc() -> "bass.Bass":
    if not _NC_CACHE:
        _NC_CACHE.append(build_nc())
    return _NC_CACHE[0]


def kernel(x, weight, a_w, b_w, magnitude):
    nc = get_nc()

    # Host: derive the folded DoRA weight (small, O(out*in) flops).
    w = weight.astype(np.float32, copy=False)
    w_eff = w + SCALING * (
        b_w.astype(np.float32, copy=False) @ a_w.astype(np.float32, copy=False)
    )
    wn = np.sqrt((w_eff.astype(np.float64) ** 2).sum(axis=1)).astype(np.float32)
    s = magnitude.astype(np.float32, copy=False).ravel() / wn
    wT_bf = np.ascontiguousarray((w_eff.T * s[None, :]).astype(ml_dtypes.bfloat16))

    # Host: shard + transpose + cast x per core.
    xf = x.reshape(M_TOT, D_IN)
    in_maps = []
    for i in range(N_CORES):
        xT_i = np.ascontiguousarray(
            xf[i * M_CORE : (i + 1) * M_CORE].T.astype(ml_dtypes.bfloat16)
        )
        in_maps.append({"xT": xT_i, "wT": wT_bf})

    trace = os.environ.get("KERNEL_TRACE", "0") == "1"
    res = run_bass_kernel_spmd(nc, in_maps, list(range(N_CORES)), trace=trace)
    if trace:
        kernel.last_result = res
    outs = [res.results[i]["out"] for i in range(N_CORES)]
    return np.concatenate(outs, axis=0).astype(np.float32).reshape(B, S, D_OUT)


# revision 5
# speedup vs baseline: 1.0068x; 1.0068x over previous
"""Trainium2 Bass kernel for nn_LoRALinear (DoRA-style LoRA linear).

Reference math:
    base = x @ W^T
    lora = sc * (x @ A^T) @ B^T          (sc = 2.0)
    w_eff = W + sc * (B @ A)
    s = magnitude / ||w_eff||_row         (row norm over in_dim)
    out = base + (s - 1) * base + s * lora
        = s * (base + lora)
        = x @ (s[:, None] * w_eff)^T

The whole op collapses to one dense matmul with a derived weight.

The derived weight (w_eff, its row norms, the DoRA scale s) depends only
on the small weight tensors, so it is folded on the host: W_fin =
(s ⊙ w_eff)^T, cast to bf16.  x is cast to bf16 and pre-transposed per
core on the host, so the device does nothing but the main matmul:
32 m-tiles x 16 accumulating bf16 matmuls (8 k-tiles x 2 n-halves),
draining PSUM to fp16 and DMAing out.  bf16 multiplies with fp32 PSUM
accumulation give ~2e-3 max rel err (tolerance 2e-2).

Data-parallel over tokens: each of the 8 cores owns 4096 of the 32768
rows.  PE floor: 512 matmuls x 512 cols / 2.4 GHz ~= 110 us/core.
"""

import os
import numpy as np
import ml_dtypes
from contextlib import ExitStack

import concourse.bass as bass
import concourse.mybir as mybir
import concourse.tile as tile
from concourse import bacc
from concourse.bass import ts
from concourse.bass_utils import run_bass_kernel_spmd

N_CORES = 8
B, S, D_IN, D_OUT, R = 4, 8192, 1024, 1024, 16
SCALING = 32.0 / 16.0
M_TOT = B * S                 # 32768 tokens
M_CORE = M_TOT // N_CORES     # 4096 tokens per core
P = 128
K_TILES = D_IN // P           # 8
MG = 512                      # tokens per x DMA group
N_GROUPS = M_CORE // MG       # 8
MT_PER_G = MG // P            # 4 m-tiles per group
NH = D_OUT // 512             # 2 n-halves of 512
F32 = mybir.dt.float32
F16 = mybir.dt.float16
BF16 = mybir.dt.bfloat16


def _x_group_ap(xT, g):
    """3D DRAM AP for x group g: [128 part (k within tile), 8 k-tiles, MG m]."""
    base = xT[:, :]
    return bass.AP(
        tensor=base.tensor,
        offset=base.offset + g * MG,
        ap=[[M_CORE, P], [P * M_CORE, K_TILES], [1, MG]],
    )


def _kernel_body(ctx: ExitStack, tc: "tile.TileContext", xT, wT, out):
    nc = tc.nc
    w_pool = ctx.enter_context(tc.tile_pool(name="w", bufs=1))
    xt_pool = ctx.enter_context(tc.tile_pool(name="xt", bufs=3))
    o_pool = ctx.enter_context(tc.tile_pool(name="o", bufs=4))
    ps_out = ctx.enter_context(tc.tile_pool(name="ps_out", bufs=4, space="PSUM"))

    # First x group first so its transfer overlaps the weight loads.
    xtgs = [None] * N_GROUPS
    xtgs[0] = xt_pool.tile([P, K_TILES, MG], BF16, tag="xt", name="xtg0")
    nc.sync.dma_start(xtgs[0][:], _x_group_ap(xT, 0))

    # Weights as one tile per k-tile so matmul k starts as soon as its
    # slice has landed (whole-tile dependency granularity).
    wks = []
    for kt in range(K_TILES):
        wk = w_pool.tile([P, D_OUT], BF16, tag=f"w{kt}", name=f"w{kt}")
        nc.sync.dma_start(wk[:], wT[ts(kt, P), :])
        wks.append(wk)

    for g in range(N_GROUPS):
        # Prefetch the next group before this group's out-DMAs hit the
        # sync queue, so stores never head-of-line-block the loads.
        if g + 1 < N_GROUPS:
            xtgs[g + 1] = xt_pool.tile(
                [P, K_TILES, MG], BF16, tag="xt", name=f"xtg{g + 1}"
            )
            nc.sync.dma_start(xtgs[g + 1][:], _x_group_ap(xT, g + 1))
        xtg = xtgs[g]

        for mt in range(MT_PER_G):
            m = g * MT_PER_G + mt
            psos = [
                ps_out.tile([P, 512], F32, tag="out", name=f"pso{h}") for h in range(NH)
            ]
            # kt outer so the very first matmul only needs w k-tile 0 and
            # this group's x; later w DMAs land while the PE streams.
            for kt in range(K_TILES):
                xsl = xtg[:, kt, ts(mt, P)]
                for h in range(NH):
                    nc.tensor.matmul(
                        psos[h][:],
                        lhsT=xsl,
                        rhs=wks[kt][:, ts(h, 512)],
                        start=(kt == 0),
                        stop=(kt == K_TILES - 1),
                    )
            o_sb = o_pool.tile([P, D_OUT], F16, tag="o")
            for h in range(NH):
                nc.scalar.copy(o_sb[:, ts(h, 512)], psos[h][:])
            nc.gpsimd.dma_start(out=out[ts(m, P), :], in_=o_sb[:])


def build_nc() -> "bass.Bass":
    nc = bacc.Bacc(
        "TRN2",
        target_bir_lowering=False,
        debug=False,
        num_devices=N_CORES,
    )
    xT = nc.dram_tensor("xT", [D_IN, M_CORE], BF16, kind="ExternalInput").ap()
    wT = nc.dram_tensor("wT", [D_IN, D_OUT], BF16, kind="ExternalInput").ap()
    out = nc.dram_tensor("out", [M_CORE, D_OUT], F16, kind="ExternalOutput").ap()

    with tile.TileContext(nc) as tc, ExitStack() as ctx:
        _kernel_body(ctx, tc, xT, wT, out)
    nc.compile()
    return nc


_NC_CACHE: list = []


def get_nc() -> "bass.Bass":
    if not _NC_CACHE:
        _NC_CACHE.append(build_nc())
    return _NC_CACHE[0]


def kernel(x, weight, a_w, b_w, magnitude):
    nc = get_nc()

    # Host: derive the folded DoRA weight (small, O(out*in) flops).
    w = weight.astype(np.float32, copy=False)
    w_eff = w + SCALING * (
        b_w.astype(np.float32, copy=False) @ a_w.astype(np.float32, copy=False)
    )
    wn = np.sqrt((w_eff.astype(np.float64) ** 2).sum(axis=1)).astype(np.float32)
    s = magnitude.astype(np.float32, copy=False).ravel() / wn
    wT_bf = np.ascontiguousarray((w_eff.T * s[None, :]).astype(ml_dtypes.bfloat16))

    # Host: shard + transpose + cast x per core.
    xf = x.reshape(M_TOT, D_IN)
    in_maps = []
    for i in range(N_CORES):
        xT_i = np.ascontiguousarray(
            xf[i * M_CORE : (i + 1) * M_CORE].T.astype(ml_dtypes.bfloat16)
        )
        in_maps.append({"xT": xT_i, "wT": wT_bf})

    trace = os.environ.get("KERNEL_TRACE", "0") == "1"
    res = run_bass_kernel_spmd(nc, in_maps, list(range(N_CORES)), trace=trace)
    if trace:
        kernel.last_result = res
    outs = [res.results[i]["out"] for i in range(N_CORES)]
    return np.concatenate(outs, axis=0).astype(np.float32).reshape(B, S, D_OUT)


# revision 23
# speedup vs baseline: 1.0201x; 1.0132x over previous
"""Trainium2 Bass kernel for nn_LoRALinear (DoRA-style LoRA linear).

Reference math:
    base = x @ W^T
    lora = sc * (x @ A^T) @ B^T          (sc = 2.0)
    w_eff = W + sc * (B @ A)
    s = magnitude / ||w_eff||_row         (row norm over in_dim)
    out = base + (s - 1) * base + s * lora
        = s * (base + lora)
        = x @ (s[:, None] * w_eff)^T

The whole op collapses to one dense matmul with a derived weight.

The derived weight (w_eff, its row norms, the DoRA scale s) depends only
on the small weight tensors, so it is folded on the host: W_fin =
(s ⊙ w_eff)^T, cast to bf16.  x is cast to bf16 and pre-transposed per
core on the host, so the device does nothing but the main matmul.
bf16 multiplies with fp32 PSUM accumulation give ~2e-3 max rel err
(tolerance 2e-2).  fp8 DoubleRow was measured at only 2x/stream and the
accuracy-passing 3-term split needs 3 streams, so bf16 single-pass wins
(PE moving-operand stream is byte-bound at 2B/cycle/lane, 2.4 GHz).

Data-parallel over tokens: each of the 8 cores owns 4096 of the 32768
rows.  Per core, 512 bf16 matmuls (32 m-tiles x 8 k-tiles x 2 n-halves)
stream gaplessly at ~220 ns each (~113 us):
  - k-outer over half-groups (2 m-tiles x 2 n-halves = 4 PSUM banks,
    double buffered) so the first matmul only needs w k-tile 0 + x
    k-tile 0 (384KB) instead of a whole group's data;
  - per-k-tile w and x tiles, DMA'd on the sync queue in consumption
    order; next x group prefetched one group ahead;
  - 7 dummy matmuls on zeroed tiles warm the PE clock (HAM gate +
    pstate ramp ~3us) during the initial DMA wait, targeting PSUM banks
    the first real accumulation group resets (start=True);
  - PSUM drains on ACT only (DVE PSUM reads contend with PE PSUM
    writes; DVE drains only the final half-group), stores issue from
    the scalar hwdge queue (gpsimd's software-DGE exit drain costs ~6us
    if it ever issues DMAs), out as fp16.
"""

import os
import numpy as np
import ml_dtypes
from contextlib import ExitStack

import concourse.bass as bass
import concourse.mybir as mybir
import concourse.tile as tile
from concourse import bacc
from concourse.bass import ts
from concourse.bass_utils import run_bass_kernel_spmd

N_CORES = 8
B, S, D_IN, D_OUT, R = 4, 8192, 1024, 1024, 16
SCALING = 32.0 / 16.0
M_TOT = B * S                 # 32768 tokens
M_CORE = M_TOT // N_CORES     # 4096 tokens per core
P = 128
K_TILES = D_IN // P           # 8
MG = 512                      # tokens per x DMA group
N_GROUPS = M_CORE // MG       # 8
MT_PER_G = MG // P            # 4 m-tiles per group
NH = D_OUT // 512             # 2 n-halves of 512
F32 = mybir.dt.float32
F16 = mybir.dt.float16
BF16 = mybir.dt.bfloat16


def _kernel_body(ctx: ExitStack, tc: "tile.TileContext", xT, wT, out):
    nc = tc.nc
    w_pool = ctx.enter_context(tc.tile_pool(name="w", bufs=1))
    xt_pool = ctx.enter_context(tc.tile_pool(name="xt", bufs=3))
    o_pool = ctx.enter_context(tc.tile_pool(name="o", bufs=8))
    ps_out = ctx.enter_context(tc.tile_pool(name="ps_out", bufs=2, space="PSUM"))

    # Per-k-tile tiles for both w and x so the k-outer pipeline can start
    # as soon as the first 384KB lands; interleave the issue order to
    # match consumption order (w_kt, x_kt alternating).
    def x_tile(g, kt):
        return xt_pool.tile([P, MG], BF16, tag=f"x{kt}", name=f"x{g}_{kt}")

    wks = []
    xks = [[None] * K_TILES for _ in range(N_GROUPS)]
    for kt in range(K_TILES):
        wk = w_pool.tile([P, D_OUT], BF16, tag=f"w{kt}", name=f"w{kt}")
        nc.sync.dma_start(wk[:], wT[ts(kt, P), :])
        wks.append(wk)
        xks[0][kt] = x_tile(0, kt)
        nc.sync.dma_start(xks[0][kt][:], xT[ts(kt, P), ts(0, MG)])

    HG = 2              # half-groups per group
    ML = MT_PER_G // HG  # m-tiles per half-group

    # Warm the PE clock (HAM gate + pstate need ~3us of continuous busy
    # to reach 2.4 GHz) during the initial DMA wait: 7 dummy matmuls
    # (~3.6us at the cold clock, ending right as the first data lands)
    # on zeroed tiles, written into PSUM banks that the first real
    # accumulation group resets anyway (start=True discards them).
    zx = w_pool.tile([P, P], BF16, tag="zx")
    nc.vector.memset(zx[:], 0.0)
    zw = w_pool.tile([P, 512], BF16, tag="zw")
    nc.vector.memset(zw[:], 0.0)
    warm = [
        ps_out.tile([P, 512], F32, tag=f"ps{ml}{h}", name=f"warm{ml}{h}")
        for ml in range(ML)
        for h in range(NH)
    ]
    for i in range(7):
        wp = warm[i % len(warm)]
        nc.tensor.matmul(wp[:], lhsT=zx[:], rhs=zw[:], start=True, stop=True)

    for g in range(N_GROUPS):
        # Prefetch next group's x on the sync queue (stores live on
        # scalar, so they never head-of-line-block these loads).
        if g + 1 < N_GROUPS:
            for kt in range(K_TILES):
                xks[g + 1][kt] = x_tile(g + 1, kt)
                nc.sync.dma_start(xks[g + 1][kt][:], xT[ts(kt, P), ts(g + 1, MG)])

        # k-outer over half-groups: 4 PSUM banks accumulate 2 m-tiles x
        # 2 n-halves across the k loop, so the first matmul only needs
        # w k-tile 0 + x k-tile 0 instead of the whole group's data.
        for hg in range(HG):
            psos = [
                [
                    ps_out.tile([P, 512], F32, tag=f"ps{ml}{h}", name=f"ps{ml}{h}")
                    for h in range(NH)
                ]
                for ml in range(ML)
            ]
            for kt in range(K_TILES):
                for ml in range(ML):
                    mt = hg * ML + ml
                    xsl = xks[g][kt][:, ts(mt, P)]
                    for h in range(NH):
                        nc.tensor.matmul(
                            psos[ml][h][:],
                            lhsT=xsl,
                            rhs=wks[kt][:, ts(h, 512)],
                            start=(kt == 0),
                            stop=(kt == K_TILES - 1),
                        )
            # DVE PSUM reads contend with PE PSUM writes (drains on DVE
            # slowed the matmul stream ~20%), so ACT drains everything
            # except the final half-group, where no matmuls remain.
            # All stores issue from the scalar (hwdge) queue: gpsimd's
            # software-DGE exit drain costs ~6us if it ever issues DMAs.
            is_last = g == N_GROUPS - 1 and hg == HG - 1
            for ml in range(ML):
                m = g * MT_PER_G + hg * ML + ml
                for h in range(NH):
                    o_sb = o_pool.tile([P, 512], F16, tag=f"o{ml}{h}")
                    if is_last and h == 1:
                        nc.vector.tensor_copy(o_sb[:], psos[ml][h][:])
                    else:
                        nc.scalar.copy(o_sb[:], psos[ml][h][:])
                    nc.scalar.dma_start(
                        out=out[ts(m, P), ts(h, 512)], in_=o_sb[:]
                    )


def build_nc() -> "bass.Bass":
    nc = bacc.Bacc(
        "TRN2",
        target_bir_lowering=False,
        debug=False,
        num_devices=N_CORES,
    )
    xT = nc.dram_tensor("xT", [D_IN, M_CORE], BF16, kind="ExternalInput").ap()
    wT = nc.dram_tensor("wT", [D_IN, D_OUT], BF16, kind="ExternalInput").ap()
    out = nc.dram_tensor("out", [M_CORE, D_OUT], F16, kind="ExternalOutput").ap()

    with tile.TileContext(nc) as tc, ExitStack() as ctx:
        _kernel_body(ctx, tc, xT, wT, out)
    nc.compile()
    return nc


_NC_CACHE: list = []


def get_nc() -> "bass.Bass":
    if not _NC_CACHE:
        _NC_CACHE.append(build_nc())
    return _NC_CACHE[0]


def kernel(x, weight, a_w, b_w, magnitude):
    nc = get_nc()
    x = np.asarray(x)
    weight = np.asarray(weight)
    a_w = np.asarray(a_w)
    b_w = np.asarray(b_w)
    magnitude = np.asarray(magnitude)

    # Host: derive the folded DoRA weight (small, O(out*in) flops).
    w = weight.astype(np.float32, copy=False)
    w_eff = w + SCALING * (
        b_w.astype(np.float32, copy=False) @ a_w.astype(np.float32, copy=False)
    )
    wn = np.sqrt((w_eff.astype(np.float64) ** 2).sum(axis=1)).astype(np.float32)
    s = magnitude.astype(np.float32, copy=False).ravel() / wn
    wT_bf = np.ascontiguousarray((w_eff.T * s[None, :]).astype(ml_dtypes.bfloat16))

    # Host: shard + transpose + cast x per core.
    xf = x.reshape(M_TOT, D_IN)
    in_maps = []
    for i in range(N_CORES):
        xT_i = np.ascontiguousarray(
            xf[i * M_CORE : (i + 1) * M_CORE].T.astype(ml_dtypes.bfloat16)
        )
        in_maps.append({"xT": xT_i, "wT": wT_bf})

    trace = os.environ.get("KERNEL_TRACE", "0") == "1"
    res = run_bass_kernel_spmd(nc, in_maps, list(range(N_CORES)), trace=trace)
    if trace:
        kernel.last_result = res
    outs = [res.results[i]["out"] for i in range(N_CORES)]
    return np.concatenate(outs, axis=0).astype(np.float32).reshape(B, S, D_OUT)
